# revision 34
# baseline (speedup 1.0000x reference)
"""Kimi-style MoE (8 routed experts top-2 + shared expert) on 8 Trainium2 cores.

Strategy: token-level expert routing is computed on the host (gate + top-k +
gather in prep; scatter/combine after), so the device kernel is pure dense
swiglu-MLP work on pre-gathered tokens. Each core runs two fixed-shape
"segments" of identical structure (up-proj [2816,1024] -> swiglu -> down-proj
[1024,1408]):

  segment A (size SA): the core's routed expert applied to that expert's
      gathered tokens (padded with zeros to SA >= max expert token count).
  segment B (size SB=T/4): one *half* of the shared expert's intermediate
      (1408 of 2816 channels -- exactly the shape of one routed expert)
      applied to a contiguous quarter of all tokens. Core c takes shared
      half c//4 and token range c%4; the two halves' partials sum on host.

This computes only the top-2-of-8 routed work (vs dense-over-E), cutting
per-core PE work from ~11.1e9 to ~5e9 MACs. All matmuls run in bf16 with
fp32 PSUM accumulation. Per-core outputs are raw segment outputs [D, S];
the host applies gate weights, down-proj biases, and the scatter-add.
"""

import sys

for _p in ("/opt/trn_rl_repo", "/opt/pypackages"):
    if _p not in sys.path:
        sys.path.insert(0, _p)

import numpy as np
import ml_dtypes

import concourse.bass as bass
import concourse.mybir as mybir
import concourse.tile as tile
from concourse import bacc
from concourse.bass import ts
from concourse.bass_utils import run_bass_kernel_spmd

BF16 = mybir.dt.bfloat16
F32 = mybir.dt.float32
NP_BF16 = ml_dtypes.bfloat16

# Problem shapes (hardcoded per the contract).
B, S, D = 2, 1024, 1024
E, TOPK = 8, 2
I = 1408
N_SHARED = 2
I_SH = N_SHARED * I          # 2816
SCALE = 2.5
T = B * S                    # 2048
P = 128
KO = D // P                  # 8 contraction subtiles
JR = I // P                  # 11 (v,g) pair tiles per segment
DT = D // P                  # 8 output partition tiles
N_CORES = 8
SB = T // 4                  # 512 tokens per shared-half segment
OUT_QUEUE = "sync"           # "sync" | "gpsimd": engine queue draining outputs
                             # (gpsimd/SWDGE measured ~10us slower on HW
                             # despite looking better in the timeline sim)


def _chunks(S_seg):
    """Split a segment's token dim into PSUM-sized (<=512) chunks."""
    n = -(-S_seg // 512)
    base = -(-S_seg // (16 * n)) * 16
    out, c0 = [], 0
    while c0 < S_seg:
        cn = min(base, S_seg - c0)
        out.append((c0, cn))
        c0 += cn
    return out


def _body(tc, io, SA, SC, pools):
    nc = tc.nc
    add = mybir.AluOpType.add
    mult = mybir.AluOpType.mult
    segs = [("a", SA), ("b", SB)] + ([("c", SC)] if SC else [])
    cpool, wpool, svpool, opool, upsum, dpsum = pools

    if True:
        xs, hs, bups, wdns = {}, {}, {}, {}
        for s, S_seg in segs:
            # x and h double-buffer across reps so the next rep's input
            # prefetch / h-writes never wait on this rep's last reads
            xs[s] = cpool.tile(
                [P, KO, S_seg], BF16, tag=f"x_{s}", name=f"x_{s}", bufs=2
            )
            hs[s] = cpool.tile(
                [P, JR, S_seg], BF16, tag=f"h_{s}", name=f"h_{s}", bufs=2
            )
            bups[s] = cpool.tile(
                [P, 2 * JR], F32, tag=f"bup_{s}", name=f"bup_{s}", bufs=2
            )
            wdns[s] = cpool.tile(
                [P, JR, DT, P], BF16, tag=f"wdn_{s}", name=f"wdn_{s}"
            )
            nc.sync.dma_start(bups[s][:], io[f"bup_{s}"][:])

        # ---- up projections + swiglu -> h (segments interleaved per j) ----
        if True:
            for j in range(JR):
                for s, S_seg in segs:
                    wtile = wpool.tile([P, KO, 2 * P], BF16, tag=f"w_{s}")
                    if j == 0:
                        # first tiles: interleave weight/x slices so the first
                        # matmuls wait on ~0.5MB, not the whole input stream
                        nc.sync.dma_start(wtile[:, :2], io[f"wup_{s}"][:, j, :2])
                        for k in range(2):
                            nc.sync.dma_start(xs[s][:, k], io[f"x_{s}"][:, k])
                        nc.sync.dma_start(wtile[:, 2:], io[f"wup_{s}"][:, j, 2:])
                        for k in range(2, KO):
                            nc.sync.dma_start(xs[s][:, k], io[f"x_{s}"][:, k])
                    else:
                        nc.sync.dma_start(wtile[:], io[f"wup_{s}"][:, j])
                    bias_v = bups[s][:, 2 * j : 2 * j + 1]
                    bias_g = bups[s][:, 2 * j + 1 : 2 * j + 2]
                    for c0, cn in _chunks(S_seg):
                        pv = upsum.tile([P, 512], F32, tag="pv")
                        pg = upsum.tile([P, 512], F32, tag="pg")
                        for k in range(KO):
                            nc.tensor.matmul(
                                pv[:, :cn], wtile[:, k, :P],
                                xs[s][:, k, c0 : c0 + cn],
                                start=(k == 0), stop=(k == KO - 1),
                            )
                        for k in range(KO):
                            nc.tensor.matmul(
                                pg[:, :cn], wtile[:, k, P:],
                                xs[s][:, k, c0 : c0 + cn],
                                start=(k == 0), stop=(k == KO - 1),
                            )
                        sv = svpool.tile([P, 512], F32, tag="sv")
                        # silu built from sigmoid to match jax numerics:
                        # sv = (v+bv) * sigmoid(v+bv); h = (g+bg) * sv
                        nc.scalar.activation(
                            sv[:, :cn], pv[:, :cn],
                            mybir.ActivationFunctionType.Sigmoid, bias=bias_v,
                        )
                        nc.vector.scalar_tensor_tensor(
                            sv[:, :cn], pv[:, :cn], bias_v, sv[:, :cn], add, mult
                        )
                        nc.vector.scalar_tensor_tensor(
                            hs[s][:, j, c0 : c0 + cn], pg[:, :cn], bias_g,
                            sv[:, :cn], add, mult,
                        )

        # down-proj weights stream in during the tail of the up phase
        for s, _ in segs:
            nc.sync.dma_start(wdns[s][:], io[f"wdn_{s}"][:])

        # ---- down projection -> out (raw, biases applied on host) ----
        if True:
            for dt in range(DT):
                for s, S_seg in segs:
                    for c0, cn in _chunks(S_seg):
                        pd = dpsum.tile([P, 512], F32, tag="pd")
                        for kd in range(JR):
                            nc.tensor.matmul(
                                pd[:, :cn], wdns[s][:, kd, dt],
                                hs[s][:, kd, c0 : c0 + cn],
                                start=(kd == 0), stop=(kd == JR - 1),
                            )
                        osb = opool.tile([P, 512], BF16, tag="osb")
                        nc.vector.tensor_copy(osb[:, :cn], pd[:, :cn])
                        eng = nc.gpsimd if OUT_QUEUE == "gpsimd" else nc.sync
                        eng.dma_start(
                            io[f"out_{s}"][ts(dt, P), c0 : c0 + cn], osb[:, :cn]
                        )


def build_nc(SA, SC, reps=1):
    nc = bacc.Bacc(None, target_bir_lowering=False, debug=False)
    io = {}
    segs = [("a", SA), ("b", SB)] + ([("c", SC)] if SC else [])
    for s, S_seg in segs:
        io[f"x_{s}"] = nc.declare_dram_parameter(
            f"x_{s}", [P, KO, S_seg], BF16, isOutput=False)
        io[f"wup_{s}"] = nc.declare_dram_parameter(
            f"wup_{s}", [P, JR, KO, 2 * P], BF16, isOutput=False)
        io[f"wdn_{s}"] = nc.declare_dram_parameter(
            f"wdn_{s}", [P, JR, DT, P], BF16, isOutput=False)
        io[f"bup_{s}"] = nc.declare_dram_parameter(
            f"bup_{s}", [P, 2 * JR], F32, isOutput=False)
        io[f"out_{s}"] = nc.declare_dram_parameter(
            f"out_{s}", [D, S_seg], BF16, isOutput=True)
    with tile.TileContext(nc) as tc:
        # SBUF pools persist across reps so same-tag tiles cycle their buffer
        # rings rep-to-rep (x/h double-buffer => cross-rep input prefetch)
        with (
            tc.tile_pool(name="const", bufs=1) as cpool,
            tc.tile_pool(name="wup", bufs=3) as wpool,
            tc.tile_pool(name="sv", bufs=4) as svpool,
            tc.tile_pool(name="outs", bufs=4) as opool,
            tc.tile_pool(name="upsum", bufs=3, space="PSUM") as upsum,
            tc.tile_pool(name="dpsum", bufs=2, space="PSUM") as dpsum,
        ):
            for _ in range(reps):
                _body(
                    tc, io, SA, SC,
                    (cpool, wpool, svpool, opool, upsum, dpsum),
                )
    nc.compile()
    return nc


# ---------------- host-side routing / pack / combine ----------------

def route(x, gate_w, gate_bias):
    """Gate on host: topk_idx [T,K], topk_w [T,K] (renormalized * SCALE)."""
    xf = x.reshape(T, D).astype(np.float32)
    logits = xf @ gate_w.T.astype(np.float32)
    scores = 1.0 / (1.0 + np.exp(-logits))
    sfc = scores + gate_bias[None, :].astype(np.float32)
    topk_idx = np.argsort(-sfc, axis=-1, kind="stable")[:, :TOPK]
    topk_w = np.take_along_axis(sfc, topk_idx, axis=-1)
    topk_w = topk_w / (topk_w.sum(-1, keepdims=True) + 1e-20) * SCALE
    return topk_idx, topk_w.astype(np.float32)


def _pack_segments(counts):
    """Pick (SA, SC): every expert's first SA tokens go to its own core's
    A-segment; overflow spills to per-core C-segments (size SC, one expert
    per C-seg, <= N_CORES of them). Minimizes PE cost (token-units, with a
    small-matmul efficiency penalty on SC)."""
    max_c = int(counts.max())
    best = None
    for SA in range(512, max(512, -(-max_c // 16) * 16) + 16, 16):
        over = [max(0, int(n) - SA) for n in counts]
        tot_over = sum(over)
        if tot_over == 0:
            cand = (float(SA), SA, 0)
        else:
            cand = None
            for SC in range(16, 513, 16):
                nsegs = sum(-(-o // SC) for o in over if o)
                if nsegs <= N_CORES:
                    eff = max(SC + 4, 68) / SC  # small-N matmul overhead
                    c = (SA + SC * eff, SA, SC)
                    if cand is None or c[0] < cand[0]:
                        cand = c
            if cand is None:
                continue
        if best is None or cand[0] < best[0]:
            best = cand
    return best[1], best[2]


def make_plan(inputs):
    """Routing plan: per-expert token lists, inverse positions, segment sizes,
    and the C-segment (overflow spill) assignment table."""
    topk_idx, topk_w = route(inputs["x"], inputs["gate_w"], inputs["gate_bias"])
    flat_e = topk_idx.reshape(-1)
    order = np.argsort(flat_e, kind="stable")
    counts = np.bincount(flat_e, minlength=E)
    starts = np.zeros(E + 1, np.int64)
    starts[1:] = np.cumsum(counts)
    tok_of = order // TOPK
    pos = np.empty(T * TOPK, np.int64)
    pos[order] = np.arange(T * TOPK) - starts[flat_e[order]]
    tok_lists = [tok_of[starts[e] : starts[e + 1]] for e in range(E)]
    SA, SC = _pack_segments(counts)
    # C-seg assignment: cseg_expert[c] = expert whose overflow chunk lives on
    # core c (or -1); cseg_core[e, j] = core holding the j-th overflow chunk.
    cseg_expert = np.full(N_CORES, -1, np.int64)
    cseg_off = np.zeros(N_CORES, np.int64)
    max_j = 1 if SC == 0 else max(1, -(-max(0, int(counts.max()) - SA) // max(SC, 1)))
    cseg_core = np.full((E, max_j), -1, np.int64)
    if SC:
        core = 0
        for e in range(E):
            o = max(0, int(counts[e]) - SA)
            j = 0
            while o > 0:
                cseg_expert[core] = e
                cseg_off[core] = SA + j * SC
                cseg_core[e, j] = core
                core += 1
                j += 1
                o -= SC
    return {
        "topk_idx": topk_idx,
        "topk_w": topk_w,
        "tok_lists": tok_lists,
        "pos": pos.reshape(T, TOPK),
        "SA": SA,
        "SC": SC,
        "cseg_expert": cseg_expert,
        "cseg_off": cseg_off,
        "cseg_core": cseg_core,
    }


def _up_layout(Wv, Wg):
    """[I,D]x2 -> [P, JR, KO, 2P] bf16 (v/g interleaved pair tiles)."""
    A = np.stack([Wv, Wg], 0).reshape(2, JR, P, KO, P)
    return np.ascontiguousarray(
        A.transpose(4, 1, 3, 0, 2).reshape(P, JR, KO, 2 * P)
    ).astype(NP_BF16)


def _dn_layout(WdT):
    """[I, D] (already transposed W2.T) -> [P, JR, DT, P] bf16."""
    return np.ascontiguousarray(
        WdT.reshape(JR, P, DT, P).transpose(1, 0, 2, 3)
    ).astype(NP_BF16)


def _bup_layout(bv, bg):
    """[I]x2 -> [P, 2*JR] f32 (v/g interleaved per pair tile)."""
    A = np.stack([bv, bg], 0).reshape(2, JR, P)
    return np.ascontiguousarray(A.transpose(2, 1, 0).reshape(P, 2 * JR)).astype(
        np.float32
    )


def _x_layout(xrows, S_seg):
    """[n, D] f32 tokens -> [P, KO, S_seg] bf16 (d on partitions), zero pad."""
    xp = np.zeros((S_seg, D), np.float32)
    xp[: len(xrows)] = xrows
    return np.ascontiguousarray(
        xp.T.reshape(KO, P, S_seg).transpose(1, 0, 2)
    ).astype(NP_BF16)


def prep_inputs(inputs, plan):
    """Full problem inputs + plan -> list of 8 per-core in_maps."""
    xf = np.asarray(inputs["x"], np.float32).reshape(T, D)
    W1 = np.asarray(inputs["W1"], np.float32)
    b1 = np.asarray(inputs["b1"], np.float32)
    W2 = np.asarray(inputs["W2"], np.float32)
    Ws1 = np.asarray(inputs["Ws1"], np.float32)
    bs1 = np.asarray(inputs["bs1"], np.float32)
    Ws2 = np.asarray(inputs["Ws2"], np.float32)
    SA, SC = plan["SA"], plan["SC"]

    in_maps = []
    for c in range(N_CORES):
        h, r = c // 4, c % 4
        hsl = slice(h * I, (h + 1) * I)
        m = {
            "x_a": _x_layout(xf[plan["tok_lists"][c][:SA]], SA),
            "x_b": _x_layout(xf[r * SB : (r + 1) * SB], SB),
            "wup_a": _up_layout(W1[c, :I], W1[c, I:]),
            "wup_b": _up_layout(Ws1[hsl], Ws1[I_SH + h * I : I_SH + (h + 1) * I]),
            "wdn_a": _dn_layout(W2[c].T),
            "wdn_b": _dn_layout(Ws2[:, hsl].T),
            "bup_a": _bup_layout(b1[c, :I], b1[c, I:]),
            "bup_b": _bup_layout(bs1[hsl], bs1[I_SH + h * I : I_SH + (h + 1) * I]),
        }
        if SC:
            e = int(plan["cseg_expert"][c])
            if e >= 0:
                off = int(plan["cseg_off"][c])
                m["x_c"] = _x_layout(xf[plan["tok_lists"][e][off : off + SC]], SC)
                m["wup_c"] = _up_layout(W1[e, :I], W1[e, I:])
                m["wdn_c"] = _dn_layout(W2[e].T)
                m["bup_c"] = _bup_layout(b1[e, :I], b1[e, I:])
            else:
                m["x_c"] = np.zeros((P, KO, SC), NP_BF16)
                m["wup_c"] = np.zeros((P, JR, KO, 2 * P), NP_BF16)
                m["wdn_c"] = np.zeros((P, JR, DT, P), NP_BF16)
                m["bup_c"] = np.zeros((P, 2 * JR), np.float32)
        in_maps.append(m)
    return in_maps


def combine_outputs(results, inputs, plan):
    """Per-core raw segment outputs -> full [B, S, D] float32 output."""
    b2 = np.asarray(inputs["b2"], np.float32)
    bs2 = np.asarray(inputs["bs2"], np.float32)
    SA, SC = plan["SA"], plan["SC"]
    out = np.zeros((T, D), np.float32)
    # shared halves: contiguous token quarters, two partials each
    for c in range(N_CORES):
        r = c % 4
        out[r * SB : (r + 1) * SB] += np.asarray(results[c]["out_b"], np.float32).T
    out += bs2[None, :]
    # routed: gather-form combine (y already excludes b2; add w*(y + b2[e]))
    # flat layout per core: [out_a (SA) | out_c (SC)]
    stride = SA + SC
    pieces = []
    for c in range(N_CORES):
        pieces.append(np.asarray(results[c]["out_a"], np.float32).T)
        if SC:
            pieces.append(np.asarray(results[c]["out_c"], np.float32).T)
    y_flat = np.concatenate(pieces, axis=0)
    topk_idx, topk_w, pos = plan["topk_idx"], plan["topk_w"], plan["pos"]
    cseg_core = plan["cseg_core"]
    for k in range(TOPK):
        e_k = topk_idx[:, k]
        p = pos[:, k]
        flat = e_k * stride + p
        if SC:
            ov = p >= SA
            if ov.any():
                q = p[ov] - SA
                core = cseg_core[e_k[ov], q // SC]
                flat[ov] = core * stride + SA + q % SC
        out += topk_w[:, k : k + 1] * (y_flat[flat] + b2[e_k])
    return np.ascontiguousarray(out.reshape(B, S, D))


_NC_CACHE = {}


def get_nc(SA, SC, reps=1):
    key = (SA, SC, reps)
    if key not in _NC_CACHE:
        _NC_CACHE[key] = build_nc(SA, SC, reps=reps)
    return _NC_CACHE[key]


def kernel(**inputs):
    plan = make_plan(inputs)
    nc = get_nc(plan["SA"], plan["SC"])
    in_maps = prep_inputs(inputs, plan)
    res = run_bass_kernel_spmd(nc, in_maps, core_ids=list(range(N_CORES)))
    return combine_outputs(res.results, inputs, plan)


if __name__ == "__main__":
    # quick self-drive (requires reference.py next to this file)
    import reference

    inputs = {k: np.asarray(v) for k, v in reference.setup_inputs().items()}
    out = kernel(**inputs)
    exp = np.asarray(reference.reference(**inputs))
    err = np.abs(out - exp).max()
    rel = err / np.abs(exp).max()
    print("absmax err:", err, "rel:", rel)


# revision 35
# speedup vs baseline: 1.2104x; 1.2104x over previous
"""Kimi-style MoE (8 routed experts top-2 + shared expert) on 8 Trainium2 cores.

Strategy: token-level expert routing is computed on the host (gate + top-k +
gather in prep; scatter/combine after), so the device kernel is pure dense
swiglu-MLP work on pre-gathered tokens. Each core runs two fixed-shape
"segments" of identical structure (up-proj [2816,1024] -> swiglu -> down-proj
[1024,1408]):

  segment A (size SA): the core's routed expert applied to that expert's
      gathered tokens (padded with zeros to SA >= max expert token count).
  segment B (size SB=T/4): one *half* of the shared expert's intermediate
      (1408 of 2816 channels -- exactly the shape of one routed expert)
      applied to a contiguous quarter of all tokens. Core c takes shared
      half c//4 and token range c%4; the two halves' partials sum on host.

This computes only the top-2-of-8 routed work (vs dense-over-E), cutting
per-core PE work from ~11.1e9 to ~5e9 MACs. All matmuls run in bf16 with
fp32 PSUM accumulation. Per-core outputs are raw segment outputs [D, S];
the host applies gate weights, down-proj biases, and the scatter-add.
"""

import sys

for _p in ("/opt/trn_rl_repo", "/opt/pypackages"):
    if _p not in sys.path:
        sys.path.insert(0, _p)

import numpy as np
import ml_dtypes

import concourse.bass as bass
import concourse.mybir as mybir
import concourse.tile as tile
from concourse import bacc
from concourse.bass import ts
from concourse.bass_utils import run_bass_kernel_spmd

BF16 = mybir.dt.bfloat16
F32 = mybir.dt.float32
NP_BF16 = ml_dtypes.bfloat16

# Problem shapes (hardcoded per the contract).
B, S, D = 2, 1024, 1024
E, TOPK = 8, 2
I = 1408
N_SHARED = 2
I_SH = N_SHARED * I          # 2816
SCALE = 2.5
T = B * S                    # 2048
P = 128
KO = D // P                  # 8 contraction subtiles
JR = I // P                  # 11 (v,g) pair tiles per segment
DT = D // P                  # 8 output partition tiles
N_CORES = 8
SB = T // 4                  # 512 tokens per shared-half segment
OUT_QUEUE = "sync"           # "sync" | "gpsimd": engine queue draining outputs
                             # (gpsimd/SWDGE measured ~10us slower on HW
                             # despite looking better in the timeline sim)


def _chunks(S_seg):
    """Split a segment's token dim into PSUM-sized (<=512) chunks."""
    n = -(-S_seg // 512)
    base = -(-S_seg // (16 * n)) * 16
    out, c0 = [], 0
    while c0 < S_seg:
        cn = min(base, S_seg - c0)
        out.append((c0, cn))
        c0 += cn
    return out


def _body(tc, io, SA, SC, pools):
    nc = tc.nc
    add = mybir.AluOpType.add
    mult = mybir.AluOpType.mult
    segs = [("a", SA), ("b", SB)] + ([("c", SC)] if SC else [])
    cpool, wpool, svpool, opool, upsum, dpsum = pools

    if True:
        xs, hs, bups, wdns = {}, {}, {}, {}
        for s, S_seg in segs:
            # x and h double-buffer across reps so the next rep's input
            # prefetch / h-writes never wait on this rep's last reads
            xs[s] = cpool.tile(
                [P, KO, S_seg], BF16, tag=f"x_{s}", name=f"x_{s}", bufs=2
            )
            hs[s] = cpool.tile(
                [P, JR, S_seg], BF16, tag=f"h_{s}", name=f"h_{s}", bufs=2
            )
            bups[s] = cpool.tile(
                [P, 2 * JR], F32, tag=f"bup_{s}", name=f"bup_{s}", bufs=2
            )
            wdns[s] = cpool.tile(
                [P, JR, DT, P], BF16, tag=f"wdn_{s}", name=f"wdn_{s}"
            )
            nc.sync.dma_start(bups[s][:], io[f"bup_{s}"][:])

        # ---- up projections + swiglu -> h (segments interleaved per j) ----
        if True:
            for j in range(JR):
                for s, S_seg in segs:
                    wtile = wpool.tile([P, KO, 2 * P], BF16, tag=f"w_{s}")
                    if j == 0:
                        # first tiles: interleave weight/x slices so the first
                        # matmuls wait on ~0.5MB, not the whole input stream
                        nc.sync.dma_start(wtile[:, :2], io[f"wup_{s}"][:, j, :2])
                        for k in range(2):
                            nc.sync.dma_start(xs[s][:, k], io[f"x_{s}"][:, k])
                        nc.sync.dma_start(wtile[:, 2:], io[f"wup_{s}"][:, j, 2:])
                        for k in range(2, KO):
                            nc.sync.dma_start(xs[s][:, k], io[f"x_{s}"][:, k])
                    else:
                        nc.sync.dma_start(wtile[:], io[f"wup_{s}"][:, j])
                    bias_v = bups[s][:, 2 * j : 2 * j + 1]
                    bias_g = bups[s][:, 2 * j + 1 : 2 * j + 2]
                    for c0, cn in _chunks(S_seg):
                        pv = upsum.tile([P, 512], F32, tag="pv")
                        pg = upsum.tile([P, 512], F32, tag="pg")
                        for k in range(KO):
                            nc.tensor.matmul(
                                pv[:, :cn], wtile[:, k, :P],
                                xs[s][:, k, c0 : c0 + cn],
                                start=(k == 0), stop=(k == KO - 1),
                            )
                        for k in range(KO):
                            nc.tensor.matmul(
                                pg[:, :cn], wtile[:, k, P:],
                                xs[s][:, k, c0 : c0 + cn],
                                start=(k == 0), stop=(k == KO - 1),
                            )
                        sv = svpool.tile([P, 512], F32, tag="sv")
                        # silu built from sigmoid to match jax numerics:
                        # sv = (v+bv) * sigmoid(v+bv); h = (g+bg) * sv
                        nc.scalar.activation(
                            sv[:, :cn], pv[:, :cn],
                            mybir.ActivationFunctionType.Sigmoid, bias=bias_v,
                        )
                        nc.vector.scalar_tensor_tensor(
                            sv[:, :cn], pv[:, :cn], bias_v, sv[:, :cn], add, mult
                        )
                        nc.vector.scalar_tensor_tensor(
                            hs[s][:, j, c0 : c0 + cn], pg[:, :cn], bias_g,
                            sv[:, :cn], add, mult,
                        )

        # down-proj weights stream in during the tail of the up phase
        for s, _ in segs:
            nc.sync.dma_start(wdns[s][:], io[f"wdn_{s}"][:])

        # ---- down projection -> out (raw, biases applied on host) ----
        if True:
            for dt in range(DT):
                for s, S_seg in segs:
                    for c0, cn in _chunks(S_seg):
                        pd = dpsum.tile([P, 512], F32, tag="pd")
                        for kd in range(JR):
                            nc.tensor.matmul(
                                pd[:, :cn], wdns[s][:, kd, dt],
                                hs[s][:, kd, c0 : c0 + cn],
                                start=(kd == 0), stop=(kd == JR - 1),
                            )
                        osb = opool.tile([P, 512], BF16, tag="osb")
                        nc.vector.tensor_copy(osb[:, :cn], pd[:, :cn])
                        eng = nc.gpsimd if OUT_QUEUE == "gpsimd" else nc.sync
                        eng.dma_start(
                            io[f"out_{s}"][ts(dt, P), c0 : c0 + cn], osb[:, :cn]
                        )


def build_nc(SA, SC, reps=1):
    nc = bacc.Bacc(None, target_bir_lowering=False, debug=False)
    io = {}
    segs = [("a", SA), ("b", SB)] + ([("c", SC)] if SC else [])
    for s, S_seg in segs:
        io[f"x_{s}"] = nc.declare_dram_parameter(
            f"x_{s}", [P, KO, S_seg], BF16, isOutput=False)
        io[f"wup_{s}"] = nc.declare_dram_parameter(
            f"wup_{s}", [P, JR, KO, 2 * P], BF16, isOutput=False)
        io[f"wdn_{s}"] = nc.declare_dram_parameter(
            f"wdn_{s}", [P, JR, DT, P], BF16, isOutput=False)
        io[f"bup_{s}"] = nc.declare_dram_parameter(
            f"bup_{s}", [P, 2 * JR], F32, isOutput=False)
        io[f"out_{s}"] = nc.declare_dram_parameter(
            f"out_{s}", [D, S_seg], BF16, isOutput=True)
    with tile.TileContext(nc) as tc:
        # SBUF pools persist across reps so same-tag tiles cycle their buffer
        # rings rep-to-rep (x/h double-buffer => cross-rep input prefetch)
        with (
            tc.tile_pool(name="const", bufs=1) as cpool,
            tc.tile_pool(name="wup", bufs=4) as wpool,
            tc.tile_pool(name="sv", bufs=4) as svpool,
            tc.tile_pool(name="outs", bufs=4) as opool,
            tc.tile_pool(name="upsum", bufs=3, space="PSUM") as upsum,
            tc.tile_pool(name="dpsum", bufs=2, space="PSUM") as dpsum,
        ):
            for _ in range(reps):
                _body(
                    tc, io, SA, SC,
                    (cpool, wpool, svpool, opool, upsum, dpsum),
                )
    nc.compile()
    return nc


# ---------------- host-side routing / pack / combine ----------------

def route(x, gate_w, gate_bias):
    """Gate on host: topk_idx [T,K], topk_w [T,K] (renormalized * SCALE)."""
    xf = x.reshape(T, D).astype(np.float32)
    logits = xf @ gate_w.T.astype(np.float32)
    scores = 1.0 / (1.0 + np.exp(-logits))
    sfc = scores + gate_bias[None, :].astype(np.float32)
    topk_idx = np.argsort(-sfc, axis=-1, kind="stable")[:, :TOPK]
    topk_w = np.take_along_axis(sfc, topk_idx, axis=-1)
    topk_w = topk_w / (topk_w.sum(-1, keepdims=True) + 1e-20) * SCALE
    return topk_idx, topk_w.astype(np.float32)


def _pack_segments(counts):
    """Pick (SA, SC): every expert's first SA tokens go to its own core's
    A-segment; overflow spills to per-core C-segments (size SC, one expert
    per C-seg, <= N_CORES of them). Minimizes PE cost (token-units, with a
    small-matmul efficiency penalty on SC)."""
    max_c = int(counts.max())
    best = None
    for SA in range(512, max(512, -(-max_c // 16) * 16) + 16, 16):
        over = [max(0, int(n) - SA) for n in counts]
        tot_over = sum(over)
        if tot_over == 0:
            cand = (float(SA), SA, 0)
        else:
            cand = None
            for SC in range(16, 513, 16):
                nsegs = sum(-(-o // SC) for o in over if o)
                if nsegs <= N_CORES:
                    eff = max(SC + 4, 68) / SC  # small-N matmul overhead
                    c = (SA + SC * eff, SA, SC)
                    if cand is None or c[0] < cand[0]:
                        cand = c
            if cand is None:
                continue
        if best is None or cand[0] < best[0]:
            best = cand
    return best[1], best[2]


def make_plan(inputs):
    """Routing plan: per-expert token lists, inverse positions, segment sizes,
    and the C-segment (overflow spill) assignment table."""
    topk_idx, topk_w = route(inputs["x"], inputs["gate_w"], inputs["gate_bias"])
    flat_e = topk_idx.reshape(-1)
    order = np.argsort(flat_e, kind="stable")
    counts = np.bincount(flat_e, minlength=E)
    starts = np.zeros(E + 1, np.int64)
    starts[1:] = np.cumsum(counts)
    tok_of = order // TOPK
    pos = np.empty(T * TOPK, np.int64)
    pos[order] = np.arange(T * TOPK) - starts[flat_e[order]]
    tok_lists = [tok_of[starts[e] : starts[e + 1]] for e in range(E)]
    SA, SC = _pack_segments(counts)
    # C-seg assignment: cseg_expert[c] = expert whose overflow chunk lives on
    # core c (or -1); cseg_core[e, j] = core holding the j-th overflow chunk.
    cseg_expert = np.full(N_CORES, -1, np.int64)
    cseg_off = np.zeros(N_CORES, np.int64)
    max_j = 1 if SC == 0 else max(1, -(-max(0, int(counts.max()) - SA) // max(SC, 1)))
    cseg_core = np.full((E, max_j), -1, np.int64)
    if SC:
        core = 0
        for e in range(E):
            o = max(0, int(counts[e]) - SA)
            j = 0
            while o > 0:
                cseg_expert[core] = e
                cseg_off[core] = SA + j * SC
                cseg_core[e, j] = core
                core += 1
                j += 1
                o -= SC
    return {
        "topk_idx": topk_idx,
        "topk_w": topk_w,
        "tok_lists": tok_lists,
        "pos": pos.reshape(T, TOPK),
        "SA": SA,
        "SC": SC,
        "cseg_expert": cseg_expert,
        "cseg_off": cseg_off,
        "cseg_core": cseg_core,
    }


def _up_layout(Wv, Wg):
    """[I,D]x2 -> [P, JR, KO, 2P] bf16 (v/g interleaved pair tiles)."""
    A = np.stack([Wv, Wg], 0).reshape(2, JR, P, KO, P)
    return np.ascontiguousarray(
        A.transpose(4, 1, 3, 0, 2).reshape(P, JR, KO, 2 * P)
    ).astype(NP_BF16)


def _dn_layout(WdT):
    """[I, D] (already transposed W2.T) -> [P, JR, DT, P] bf16."""
    return np.ascontiguousarray(
        WdT.reshape(JR, P, DT, P).transpose(1, 0, 2, 3)
    ).astype(NP_BF16)


def _bup_layout(bv, bg):
    """[I]x2 -> [P, 2*JR] f32 (v/g interleaved per pair tile)."""
    A = np.stack([bv, bg], 0).reshape(2, JR, P)
    return np.ascontiguousarray(A.transpose(2, 1, 0).reshape(P, 2 * JR)).astype(
        np.float32
    )


def _x_layout(xrows, S_seg):
    """[n, D] f32 tokens -> [P, KO, S_seg] bf16 (d on partitions), zero pad."""
    xp = np.zeros((S_seg, D), np.float32)
    xp[: len(xrows)] = xrows
    return np.ascontiguousarray(
        xp.T.reshape(KO, P, S_seg).transpose(1, 0, 2)
    ).astype(NP_BF16)


def prep_inputs(inputs, plan):
    """Full problem inputs + plan -> list of 8 per-core in_maps."""
    xf = np.asarray(inputs["x"], np.float32).reshape(T, D)
    W1 = np.asarray(inputs["W1"], np.float32)
    b1 = np.asarray(inputs["b1"], np.float32)
    W2 = np.asarray(inputs["W2"], np.float32)
    Ws1 = np.asarray(inputs["Ws1"], np.float32)
    bs1 = np.asarray(inputs["bs1"], np.float32)
    Ws2 = np.asarray(inputs["Ws2"], np.float32)
    SA, SC = plan["SA"], plan["SC"]

    in_maps = []
    for c in range(N_CORES):
        h, r = c // 4, c % 4
        hsl = slice(h * I, (h + 1) * I)
        m = {
            "x_a": _x_layout(xf[plan["tok_lists"][c][:SA]], SA),
            "x_b": _x_layout(xf[r * SB : (r + 1) * SB], SB),
            "wup_a": _up_layout(W1[c, :I], W1[c, I:]),
            "wup_b": _up_layout(Ws1[hsl], Ws1[I_SH + h * I : I_SH + (h + 1) * I]),
            "wdn_a": _dn_layout(W2[c].T),
            "wdn_b": _dn_layout(Ws2[:, hsl].T),
            "bup_a": _bup_layout(b1[c, :I], b1[c, I:]),
            "bup_b": _bup_layout(bs1[hsl], bs1[I_SH + h * I : I_SH + (h + 1) * I]),
        }
        if SC:
            e = int(plan["cseg_expert"][c])
            if e >= 0:
                off = int(plan["cseg_off"][c])
                m["x_c"] = _x_layout(xf[plan["tok_lists"][e][off : off + SC]], SC)
                m["wup_c"] = _up_layout(W1[e, :I], W1[e, I:])
                m["wdn_c"] = _dn_layout(W2[e].T)
                m["bup_c"] = _bup_layout(b1[e, :I], b1[e, I:])
            else:
                m["x_c"] = np.zeros((P, KO, SC), NP_BF16)
                m["wup_c"] = np.zeros((P, JR, KO, 2 * P), NP_BF16)
                m["wdn_c"] = np.zeros((P, JR, DT, P), NP_BF16)
                m["bup_c"] = np.zeros((P, 2 * JR), np.float32)
        in_maps.append(m)
    return in_maps


def combine_outputs(results, inputs, plan):
    """Per-core raw segment outputs -> full [B, S, D] float32 output."""
    b2 = np.asarray(inputs["b2"], np.float32)
    bs2 = np.asarray(inputs["bs2"], np.float32)
    SA, SC = plan["SA"], plan["SC"]
    out = np.zeros((T, D), np.float32)
    # shared halves: contiguous token quarters, two partials each
    for c in range(N_CORES):
        r = c % 4
        out[r * SB : (r + 1) * SB] += np.asarray(results[c]["out_b"], np.float32).T
    out += bs2[None, :]
    # routed: gather-form combine (y already excludes b2; add w*(y + b2[e]))
    # flat layout per core: [out_a (SA) | out_c (SC)]
    stride = SA + SC
    pieces = []
    for c in range(N_CORES):
        pieces.append(np.asarray(results[c]["out_a"], np.float32).T)
        if SC:
            pieces.append(np.asarray(results[c]["out_c"], np.float32).T)
    y_flat = np.concatenate(pieces, axis=0)
    topk_idx, topk_w, pos = plan["topk_idx"], plan["topk_w"], plan["pos"]
    cseg_core = plan["cseg_core"]
    for k in range(TOPK):
        e_k = topk_idx[:, k]
        p = pos[:, k]
        flat = e_k * stride + p
        if SC:
            ov = p >= SA
            if ov.any():
                q = p[ov] - SA
                core = cseg_core[e_k[ov], q // SC]
                flat[ov] = core * stride + SA + q % SC
        out += topk_w[:, k : k + 1] * (y_flat[flat] + b2[e_k])
    return np.ascontiguousarray(out.reshape(B, S, D))


_NC_CACHE = {}


def get_nc(SA, SC, reps=1):
    key = (SA, SC, reps)
    if key not in _NC_CACHE:
        _NC_CACHE[key] = build_nc(SA, SC, reps=reps)
    return _NC_CACHE[key]


def kernel(**inputs):
    plan = make_plan(inputs)
    nc = get_nc(plan["SA"], plan["SC"])
    in_maps = prep_inputs(inputs, plan)
    res = run_bass_kernel_spmd(nc, in_maps, core_ids=list(range(N_CORES)))
    return combine_outputs(res.results, inputs, plan)


if __name__ == "__main__":
    # quick self-drive (requires reference.py next to this file)
    import reference

    inputs = {k: np.asarray(v) for k, v in reference.setup_inputs().items()}
    out = kernel(**inputs)
    exp = np.asarray(reference.reference(**inputs))
    err = np.abs(out - exp).max()
    rel = err / np.abs(exp).max()
    print("absmax err:", err, "rel:", rel)


# revision 39
# speedup vs baseline: 1.2420x; 1.0261x over previous
"""Kimi-style MoE (8 routed experts top-2 + shared expert) on 8 Trainium2 cores.

Strategy: token-level expert routing is computed on the host (gate + top-k +
gather in prep; scatter/combine after), so the device kernel is pure dense
swiglu-MLP work on pre-gathered tokens. Each core runs two fixed-shape
"segments" of identical structure (up-proj [2816,1024] -> swiglu -> down-proj
[1024,1408]):

  segment A (size SA): the core's routed expert applied to that expert's
      gathered tokens (padded with zeros to SA >= max expert token count).
  segment B (size SB=T/4): one *half* of the shared expert's intermediate
      (1408 of 2816 channels -- exactly the shape of one routed expert)
      applied to a contiguous quarter of all tokens. Core c takes shared
      half c//4 and token range c%4; the two halves' partials sum on host.

This computes only the top-2-of-8 routed work (vs dense-over-E), cutting
per-core PE work from ~11.1e9 to ~5e9 MACs. All matmuls run in bf16 with
fp32 PSUM accumulation. Per-core outputs are raw segment outputs [D, S];
the host applies gate weights, down-proj biases, and the scatter-add.
"""

import sys

for _p in ("/opt/trn_rl_repo", "/opt/pypackages"):
    if _p not in sys.path:
        sys.path.insert(0, _p)

import numpy as np
import ml_dtypes

import concourse.bass as bass
import concourse.mybir as mybir
import concourse.tile as tile
from concourse import bacc
from concourse.bass import ts
from concourse.bass_utils import run_bass_kernel_spmd

BF16 = mybir.dt.bfloat16
F32 = mybir.dt.float32
NP_BF16 = ml_dtypes.bfloat16

# Problem shapes (hardcoded per the contract).
B, S, D = 2, 1024, 1024
E, TOPK = 8, 2
I = 1408
N_SHARED = 2
I_SH = N_SHARED * I          # 2816
SCALE = 2.5
T = B * S                    # 2048
P = 128
KO = D // P                  # 8 contraction subtiles
JR = I // P                  # 11 (v,g) pair tiles per segment
DT = D // P                  # 8 output partition tiles
N_CORES = 8
SB = T // 4                  # 512 tokens per shared-half segment
OUT_QUEUE = "sync"           # "sync" | "gpsimd": engine queue draining outputs
                             # (gpsimd/SWDGE measured ~10us slower on HW
                             # despite looking better in the timeline sim)


def _chunks(S_seg):
    """Split a segment's token dim into PSUM-sized (<=512) chunks."""
    n = -(-S_seg // 512)
    base = -(-S_seg // (16 * n)) * 16
    out, c0 = [], 0
    while c0 < S_seg:
        cn = min(base, S_seg - c0)
        out.append((c0, cn))
        c0 += cn
    return out


def _body(tc, io, SA, SC, pools):
    nc = tc.nc
    add = mybir.AluOpType.add
    mult = mybir.AluOpType.mult
    segs = [("a", SA), ("b", SB)] + ([("c", SC)] if SC else [])
    cpool, wpool, svpool, opool, upsum, dpsum = pools

    if True:
        xs, hs, bups, wdns = {}, {}, {}, {}
        for s, S_seg in segs:
            # x and h double-buffer across reps so the next rep's input
            # prefetch / h-writes never wait on this rep's last reads
            xs[s] = cpool.tile(
                [P, KO, S_seg], BF16, tag=f"x_{s}", name=f"x_{s}", bufs=2
            )
            hs[s] = cpool.tile(
                [P, JR, S_seg], BF16, tag=f"h_{s}", name=f"h_{s}", bufs=2
            )
            bups[s] = cpool.tile(
                [P, 2 * JR], F32, tag=f"bup_{s}", name=f"bup_{s}", bufs=2
            )
            wdns[s] = cpool.tile(
                [P, JR, DT, P], BF16, tag=f"wdn_{s}", name=f"wdn_{s}"
            )
            nc.sync.dma_start(bups[s][:], io[f"bup_{s}"][:])

        # ---- up projections + swiglu -> h (segments interleaved per j) ----
        if True:
            for j in range(JR):
                wtiles = {}
                for s, S_seg in segs:
                    wtile = wpool.tile([P, KO, 2 * P], BF16, tag=f"w_{s}")
                    if j == 0:
                        # first tiles: interleave weight/x slices so the first
                        # matmuls wait on ~0.5MB, not the whole input stream
                        nc.sync.dma_start(wtile[:, :2], io[f"wup_{s}"][:, j, :2])
                        for k in range(2):
                            nc.sync.dma_start(xs[s][:, k], io[f"x_{s}"][:, k])
                        nc.sync.dma_start(wtile[:, 2:], io[f"wup_{s}"][:, j, 2:])
                        for k in range(2, KO):
                            nc.sync.dma_start(xs[s][:, k], io[f"x_{s}"][:, k])
                    else:
                        nc.sync.dma_start(wtile[:], io[f"wup_{s}"][:, j])
                    wtiles[s] = wtile

                # segment c's short (N=SC) matmuls interleave 1:1 with
                # segment a's long N=512 ones so each short matmul's
                # LDWEIGHTS is pulled ahead during a long stream instead of
                # stalling back-to-back; b runs alone as before.
                plans = [("a", SA, "c" if SC else None)] + [("b", SB, None)]
                for s, S_seg, mate in plans:
                    bias_v = bups[s][:, 2 * j : 2 * j + 1]
                    bias_g = bups[s][:, 2 * j + 1 : 2 * j + 2]
                    for ci, (c0, cn) in enumerate(_chunks(S_seg)):
                        lead = mate if ci == 0 else None  # mate rides chunk 0
                        pv = upsum.tile([P, 512], F32, tag="pv", bufs=3)
                        pg = upsum.tile([P, 512], F32, tag="pg", bufs=2)
                        if lead:
                            pvm = upsum.tile([P, 512], F32, tag="pv", bufs=3)
                            pgm = upsum.tile([P, 512], F32, tag="pg", bufs=2)
                        for k in range(KO):
                            nc.tensor.matmul(
                                pv[:, :cn], wtiles[s][:, k, :P],
                                xs[s][:, k, c0 : c0 + cn],
                                start=(k == 0), stop=(k == KO - 1),
                            )
                            if lead:
                                nc.tensor.matmul(
                                    pvm[:, :SC], wtiles[lead][:, k, :P],
                                    xs[lead][:, k, :SC],
                                    start=(k == 0), stop=(k == KO - 1),
                                )
                        for k in range(KO):
                            nc.tensor.matmul(
                                pg[:, :cn], wtiles[s][:, k, P:],
                                xs[s][:, k, c0 : c0 + cn],
                                start=(k == 0), stop=(k == KO - 1),
                            )
                            if lead:
                                nc.tensor.matmul(
                                    pgm[:, :SC], wtiles[lead][:, k, P:],
                                    xs[lead][:, k, :SC],
                                    start=(k == 0), stop=(k == KO - 1),
                                )
                        emits = [(s, pv, pg, c0, cn, bias_v, bias_g)]
                        if lead:
                            emits.append((
                                lead, pvm, pgm, 0, SC,
                                bups[lead][:, 2 * j : 2 * j + 1],
                                bups[lead][:, 2 * j + 1 : 2 * j + 2],
                            ))
                        for es, epv, epg, ec0, ecn, ebv, ebg in emits:
                            sv = svpool.tile([P, 512], F32, tag="sv")
                            # silu built from sigmoid to match jax numerics:
                            # sv = (v+bv)*sigmoid(v+bv); h = (g+bg)*sv
                            nc.scalar.activation(
                                sv[:, :ecn], epv[:, :ecn],
                                mybir.ActivationFunctionType.Sigmoid, bias=ebv,
                            )
                            nc.vector.scalar_tensor_tensor(
                                sv[:, :ecn], epv[:, :ecn], ebv, sv[:, :ecn],
                                add, mult,
                            )
                            nc.vector.scalar_tensor_tensor(
                                hs[es][:, j, ec0 : ec0 + ecn], epg[:, :ecn],
                                ebg, sv[:, :ecn], add, mult,
                            )

        # down-proj weights stream in during the tail of the up phase
        for s, _ in segs:
            nc.sync.dma_start(wdns[s][:], io[f"wdn_{s}"][:])

        # ---- down projection -> out (raw, biases applied on host) ----
        # segment c's short chain interleaves with a's (same LDWEIGHTS-hiding
        # trick as the up phase)
        eng = nc.gpsimd if OUT_QUEUE == "gpsimd" else nc.sync
        if True:
            for dt in range(DT):
                plans = [("a", SA, "c" if SC else None)] + [("b", SB, None)]
                for s, S_seg, mate in plans:
                    for ci, (c0, cn) in enumerate(_chunks(S_seg)):
                        lead = mate if ci == 0 else None
                        pd = dpsum.tile([P, 512], F32, tag="pd", bufs=3)
                        if lead:
                            pdm = dpsum.tile([P, 512], F32, tag="pd", bufs=3)
                        for kd in range(JR):
                            nc.tensor.matmul(
                                pd[:, :cn], wdns[s][:, kd, dt],
                                hs[s][:, kd, c0 : c0 + cn],
                                start=(kd == 0), stop=(kd == JR - 1),
                            )
                            if lead:
                                nc.tensor.matmul(
                                    pdm[:, :SC], wdns[lead][:, kd, dt],
                                    hs[lead][:, kd, :SC],
                                    start=(kd == 0), stop=(kd == JR - 1),
                                )
                        emits = [(s, pd, c0, cn)]
                        if lead:
                            emits.append((lead, pdm, 0, SC))
                        for es, epd, ec0, ecn in emits:
                            osb = opool.tile([P, 512], BF16, tag="osb")
                            nc.vector.tensor_copy(osb[:, :ecn], epd[:, :ecn])
                            eng.dma_start(
                                io[f"out_{es}"][ts(dt, P), ec0 : ec0 + ecn],
                                osb[:, :ecn],
                            )


def build_nc(SA, SC, reps=1):
    nc = bacc.Bacc(None, target_bir_lowering=False, debug=False)
    io = {}
    segs = [("a", SA), ("b", SB)] + ([("c", SC)] if SC else [])
    for s, S_seg in segs:
        io[f"x_{s}"] = nc.declare_dram_parameter(
            f"x_{s}", [P, KO, S_seg], BF16, isOutput=False)
        io[f"wup_{s}"] = nc.declare_dram_parameter(
            f"wup_{s}", [P, JR, KO, 2 * P], BF16, isOutput=False)
        io[f"wdn_{s}"] = nc.declare_dram_parameter(
            f"wdn_{s}", [P, JR, DT, P], BF16, isOutput=False)
        io[f"bup_{s}"] = nc.declare_dram_parameter(
            f"bup_{s}", [P, 2 * JR], F32, isOutput=False)
        io[f"out_{s}"] = nc.declare_dram_parameter(
            f"out_{s}", [D, S_seg], BF16, isOutput=True)
    with tile.TileContext(nc) as tc:
        # SBUF pools persist across reps so same-tag tiles cycle their buffer
        # rings rep-to-rep (x/h double-buffer => cross-rep input prefetch)
        with (
            tc.tile_pool(name="const", bufs=1) as cpool,
            tc.tile_pool(name="wup", bufs=4) as wpool,
            tc.tile_pool(name="sv", bufs=4) as svpool,
            tc.tile_pool(name="outs", bufs=4) as opool,
            tc.tile_pool(name="upsum", bufs=3, space="PSUM") as upsum,
            tc.tile_pool(name="dpsum", bufs=2, space="PSUM") as dpsum,
        ):
            for _ in range(reps):
                _body(
                    tc, io, SA, SC,
                    (cpool, wpool, svpool, opool, upsum, dpsum),
                )
    nc.compile()
    return nc


# ---------------- host-side routing / pack / combine ----------------

def route(x, gate_w, gate_bias):
    """Gate on host: topk_idx [T,K], topk_w [T,K] (renormalized * SCALE)."""
    xf = x.reshape(T, D).astype(np.float32)
    logits = xf @ gate_w.T.astype(np.float32)
    scores = 1.0 / (1.0 + np.exp(-logits))
    sfc = scores + gate_bias[None, :].astype(np.float32)
    topk_idx = np.argsort(-sfc, axis=-1, kind="stable")[:, :TOPK]
    topk_w = np.take_along_axis(sfc, topk_idx, axis=-1)
    topk_w = topk_w / (topk_w.sum(-1, keepdims=True) + 1e-20) * SCALE
    return topk_idx, topk_w.astype(np.float32)


def _pack_segments(counts):
    """Pick (SA, SC): every expert's first SA tokens go to its own core's
    A-segment; overflow spills to per-core C-segments (size SC, one expert
    per C-seg, <= N_CORES of them). Minimizes PE cost (token-units, with a
    small-matmul efficiency penalty on SC)."""
    max_c = int(counts.max())
    best = None
    for SA in range(512, max(512, -(-max_c // 16) * 16) + 16, 16):
        over = [max(0, int(n) - SA) for n in counts]
        tot_over = sum(over)
        if tot_over == 0:
            cand = (float(SA), SA, 0)
        else:
            cand = None
            for SC in range(16, 513, 16):
                nsegs = sum(-(-o // SC) for o in over if o)
                if nsegs <= N_CORES:
                    eff = max(SC + 4, 68) / SC  # small-N matmul overhead
                    c = (SA + SC * eff, SA, SC)
                    if cand is None or c[0] < cand[0]:
                        cand = c
            if cand is None:
                continue
        if best is None or cand[0] < best[0]:
            best = cand
    return best[1], best[2]


def make_plan(inputs):
    """Routing plan: per-expert token lists, inverse positions, segment sizes,
    and the C-segment (overflow spill) assignment table."""
    topk_idx, topk_w = route(inputs["x"], inputs["gate_w"], inputs["gate_bias"])
    flat_e = topk_idx.reshape(-1)
    order = np.argsort(flat_e, kind="stable")
    counts = np.bincount(flat_e, minlength=E)
    starts = np.zeros(E + 1, np.int64)
    starts[1:] = np.cumsum(counts)
    tok_of = order // TOPK
    pos = np.empty(T * TOPK, np.int64)
    pos[order] = np.arange(T * TOPK) - starts[flat_e[order]]
    tok_lists = [tok_of[starts[e] : starts[e + 1]] for e in range(E)]
    SA, SC = _pack_segments(counts)
    # C-seg assignment: cseg_expert[c] = expert whose overflow chunk lives on
    # core c (or -1); cseg_core[e, j] = core holding the j-th overflow chunk.
    cseg_expert = np.full(N_CORES, -1, np.int64)
    cseg_off = np.zeros(N_CORES, np.int64)
    max_j = 1 if SC == 0 else max(1, -(-max(0, int(counts.max()) - SA) // max(SC, 1)))
    cseg_core = np.full((E, max_j), -1, np.int64)
    if SC:
        core = 0
        for e in range(E):
            o = max(0, int(counts[e]) - SA)
            j = 0
            while o > 0:
                cseg_expert[core] = e
                cseg_off[core] = SA + j * SC
                cseg_core[e, j] = core
                core += 1
                j += 1
                o -= SC
    return {
        "topk_idx": topk_idx,
        "topk_w": topk_w,
        "tok_lists": tok_lists,
        "pos": pos.reshape(T, TOPK),
        "SA": SA,
        "SC": SC,
        "cseg_expert": cseg_expert,
        "cseg_off": cseg_off,
        "cseg_core": cseg_core,
    }


def _up_layout(Wv, Wg):
    """[I,D]x2 -> [P, JR, KO, 2P] bf16 (v/g interleaved pair tiles)."""
    A = np.stack([Wv, Wg], 0).reshape(2, JR, P, KO, P)
    return np.ascontiguousarray(
        A.transpose(4, 1, 3, 0, 2).reshape(P, JR, KO, 2 * P)
    ).astype(NP_BF16)


def _dn_layout(WdT):
    """[I, D] (already transposed W2.T) -> [P, JR, DT, P] bf16."""
    return np.ascontiguousarray(
        WdT.reshape(JR, P, DT, P).transpose(1, 0, 2, 3)
    ).astype(NP_BF16)


def _bup_layout(bv, bg):
    """[I]x2 -> [P, 2*JR] f32 (v/g interleaved per pair tile)."""
    A = np.stack([bv, bg], 0).reshape(2, JR, P)
    return np.ascontiguousarray(A.transpose(2, 1, 0).reshape(P, 2 * JR)).astype(
        np.float32
    )


def _x_layout(xrows, S_seg):
    """[n, D] f32 tokens -> [P, KO, S_seg] bf16 (d on partitions), zero pad."""
    xp = np.zeros((S_seg, D), np.float32)
    xp[: len(xrows)] = xrows
    return np.ascontiguousarray(
        xp.T.reshape(KO, P, S_seg).transpose(1, 0, 2)
    ).astype(NP_BF16)


def prep_inputs(inputs, plan):
    """Full problem inputs + plan -> list of 8 per-core in_maps."""
    xf = np.asarray(inputs["x"], np.float32).reshape(T, D)
    W1 = np.asarray(inputs["W1"], np.float32)
    b1 = np.asarray(inputs["b1"], np.float32)
    W2 = np.asarray(inputs["W2"], np.float32)
    Ws1 = np.asarray(inputs["Ws1"], np.float32)
    bs1 = np.asarray(inputs["bs1"], np.float32)
    Ws2 = np.asarray(inputs["Ws2"], np.float32)
    SA, SC = plan["SA"], plan["SC"]

    in_maps = []
    for c in range(N_CORES):
        h, r = c // 4, c % 4
        hsl = slice(h * I, (h + 1) * I)
        m = {
            "x_a": _x_layout(xf[plan["tok_lists"][c][:SA]], SA),
            "x_b": _x_layout(xf[r * SB : (r + 1) * SB], SB),
            "wup_a": _up_layout(W1[c, :I], W1[c, I:]),
            "wup_b": _up_layout(Ws1[hsl], Ws1[I_SH + h * I : I_SH + (h + 1) * I]),
            "wdn_a": _dn_layout(W2[c].T),
            "wdn_b": _dn_layout(Ws2[:, hsl].T),
            "bup_a": _bup_layout(b1[c, :I], b1[c, I:]),
            "bup_b": _bup_layout(bs1[hsl], bs1[I_SH + h * I : I_SH + (h + 1) * I]),
        }
        if SC:
            e = int(plan["cseg_expert"][c])
            if e >= 0:
                off = int(plan["cseg_off"][c])
                m["x_c"] = _x_layout(xf[plan["tok_lists"][e][off : off + SC]], SC)
                m["wup_c"] = _up_layout(W1[e, :I], W1[e, I:])
                m["wdn_c"] = _dn_layout(W2[e].T)
                m["bup_c"] = _bup_layout(b1[e, :I], b1[e, I:])
            else:
                m["x_c"] = np.zeros((P, KO, SC), NP_BF16)
                m["wup_c"] = np.zeros((P, JR, KO, 2 * P), NP_BF16)
                m["wdn_c"] = np.zeros((P, JR, DT, P), NP_BF16)
                m["bup_c"] = np.zeros((P, 2 * JR), np.float32)
        in_maps.append(m)
    return in_maps


def combine_outputs(results, inputs, plan):
    """Per-core raw segment outputs -> full [B, S, D] float32 output."""
    b2 = np.asarray(inputs["b2"], np.float32)
    bs2 = np.asarray(inputs["bs2"], np.float32)
    SA, SC = plan["SA"], plan["SC"]
    out = np.zeros((T, D), np.float32)
    # shared halves: contiguous token quarters, two partials each
    for c in range(N_CORES):
        r = c % 4
        out[r * SB : (r + 1) * SB] += np.asarray(results[c]["out_b"], np.float32).T
    out += bs2[None, :]
    # routed: gather-form combine (y already excludes b2; add w*(y + b2[e]))
    # flat layout per core: [out_a (SA) | out_c (SC)]
    stride = SA + SC
    pieces = []
    for c in range(N_CORES):
        pieces.append(np.asarray(results[c]["out_a"], np.float32).T)
        if SC:
            pieces.append(np.asarray(results[c]["out_c"], np.float32).T)
    y_flat = np.concatenate(pieces, axis=0)
    topk_idx, topk_w, pos = plan["topk_idx"], plan["topk_w"], plan["pos"]
    cseg_core = plan["cseg_core"]
    for k in range(TOPK):
        e_k = topk_idx[:, k]
        p = pos[:, k]
        flat = e_k * stride + p
        if SC:
            ov = p >= SA
            if ov.any():
                q = p[ov] - SA
                core = cseg_core[e_k[ov], q // SC]
                flat[ov] = core * stride + SA + q % SC
        out += topk_w[:, k : k + 1] * (y_flat[flat] + b2[e_k])
    return np.ascontiguousarray(out.reshape(B, S, D))


_NC_CACHE = {}


def get_nc(SA, SC, reps=1):
    key = (SA, SC, reps)
    if key not in _NC_CACHE:
        _NC_CACHE[key] = build_nc(SA, SC, reps=reps)
    return _NC_CACHE[key]


def kernel(**inputs):
    plan = make_plan(inputs)
    nc = get_nc(plan["SA"], plan["SC"])
    in_maps = prep_inputs(inputs, plan)
    res = run_bass_kernel_spmd(nc, in_maps, core_ids=list(range(N_CORES)))
    return combine_outputs(res.results, inputs, plan)


if __name__ == "__main__":
    # quick self-drive (requires reference.py next to this file)
    import reference

    inputs = {k: np.asarray(v) for k, v in reference.setup_inputs().items()}
    out = kernel(**inputs)
    exp = np.asarray(reference.reference(**inputs))
    err = np.abs(out - exp).max()
    rel = err / np.abs(exp).max()
    print("absmax err:", err, "rel:", rel)
